# revision 59
# baseline (speedup 1.0000x reference)
"""Multi-head GAT layer on 8 Trainium2 NeuronCores (Bass/Tile SPMD kernel).

Strategy (edge-parallel, target-sharded):
  - Edges sorted by target, sharded across 8 cores by contiguous target
    ranges (N/8 nodes each): softmax + aggregation are core-local.
  - Phase 1a (replicated on every core): one bf16 PE pass over the node
    features builds a per-node table row [ h (128) | s2 (8) | s1 (8) ]
    (bf16, 512B rows) where h = NF @ W.T + b and s1/s2 are the per-node
    attention scores h . a1 / h . a2 (fused into the same matmul via
    W.T @ A12).
  - Phase 1b (per-core data, same program): the core's own 6250 target
    rows are recomputed into a resident SBUF table (fp32, including
    degree) so phase 2 needs no self-row gather at all.
  - Phase 2, software-pipelined per 128-target block:
      A: edge slots (padded to 128-slot tiles, sorted by src) fetched
         with dma_gather (int16 indices + static per-group base, 4 SWDGE
         queues, 40KB descriptor rings); both one-hot orientations
         (slot->target for accumulation, target->slot for the s1 gather)
         stream in as host-built fp8 matmul weights, so no on-device
         one-hot construction or PE transposes are needed; s1-per-slot
         comes from small PE matmuls against the transposed one-hot;
      D1: scores z = s1e+s2 and lrelu on DVE, exp on ACT (the only ACT
          LUT), ex expanded across F_OUT on ACT so the big DVE multiply
          runs on contiguous operands;
      D2: weighted messages and one PE matmul per tile accumulating
          [Msg | ex] into PSUM (fp8 one-hot lhsT x bf16 rhs);
      E: per-block PSUM drain to SBUF on ACT, then a batched tail every
         4 blocks (softmax division, degree-prescaled skip term from the
         SBUF own-table, ELU as max(x, exp(min(x,0))-1), batched output
         write).
    Stages are emitted skewed (fetch i, D1 i-2, D2 i-3, drain i-4, tails
    3 further back) so each in-order engine queue interleaves
    independent blocks instead of serializing on the per-block
    dependency chain.
"""

import numpy as np

N_CORES = 8
_last_results = None  # BassKernelResults of the most recent run (for harnesses)


def _install_ntff_hook():
    """Register the axon NTFF profiling hook if the image lacks antenv.axon_hooks."""
    import sys, types
    try:
        from antenv.axon_hooks import get_axon_ntff_profile_hook  # noqa: F401
        return
    except ImportError:
        pass
    try:
        mod = types.ModuleType("antenv.axon_hooks")
        holder = [None]
        mod.set_axon_ntff_profile_hook = lambda h: holder.__setitem__(0, h)
        mod.get_axon_ntff_profile_hook = lambda: holder[0]
        sys.modules["antenv.axon_hooks"] = mod
        from trn_agent_boot.trn_boot import _ntff_profile_via_ctypes
        mod.set_axon_ntff_profile_hook(
            _ntff_profile_via_ctypes("/opt/axon/libaxon_pjrt.so"))
    except Exception:
        sys.modules.pop("antenv.axon_hooks", None)


def kernel(node_features, edge_index, W, b, a):
    return gat_multicore(
        np.asarray(node_features, dtype=np.float32),
        np.asarray(edge_index, dtype=np.int32),
        np.asarray(W, dtype=np.float32),
        np.asarray(b, dtype=np.float32),
        np.asarray(a, dtype=np.float32),
    )


def gat_multicore(nf, ei, W, b, a, slope=0.2):
    import sys
    if "/opt/trn_rl_repo" not in sys.path:
        sys.path.insert(0, "/opt/trn_rl_repo")
    import ml_dtypes
    import concourse.bacc as bacc
    import concourse.tile as tile
    import concourse.mybir as mybir
    from concourse import library_config
    from concourse.bass_utils import run_bass_kernel_spmd
    from contextlib import ExitStack

    fp32 = mybir.dt.float32
    bf16 = mybir.dt.bfloat16
    i16 = mybir.dt.int16
    AF = mybir.ActivationFunctionType
    OP = mybir.AluOpType
    bfnp = ml_dtypes.bfloat16

    N, F_IN = nf.shape
    E = ei.shape[1]
    HF = W.shape[0]               # H * F_OUT
    F_OUT = a.shape[0] // 2
    H = HF // F_OUT
    assert F_IN == 128 and HF == 128, "kernel assumes 128 in/out features"
    assert N % N_CORES == 0
    NPC = N // N_CORES            # targets per core
    NBLK = (NPC + 127) // 128     # 128-target blocks per core
    GRP = 8                       # max tiles per gather group
    ROW = 256                     # fp8 elements per table row (256 B)
    SPAN = 30000                  # max int16 index span per gather group

    # ---------------- host prep: weights ----------------
    WT = np.ascontiguousarray(W.T)                       # [F_IN, HF]
    # A12 column order: [s2 (a2) | s1 (a1)] to match the table row layout
    A12 = np.zeros((HF, 2 * H), dtype=np.float32)
    for hd in range(H):
        A12[hd * F_OUT:(hd + 1) * F_OUT, hd] = a[F_OUT:]        # s2
        A12[hd * F_OUT:(hd + 1) * F_OUT, H + hd] = a[:F_OUT]    # s1
    M12 = (WT @ A12).astype(np.float32)                  # [F_IN, 2H]
    b12 = (b @ A12).astype(np.float32)                   # [2H]
    b_ext = np.concatenate([b, b12]).astype(np.float32)  # [144]
    b_rep = np.broadcast_to(b_ext, (128, HF + 2 * H)).copy()
    NFT = np.ascontiguousarray(nf.T).astype(bfnp)        # [F_IN, N] bf16

    # ---------------- host prep: graph structure ----------------
    src, tgt = ei[0].astype(np.int64), ei[1].astype(np.int64)
    order = np.argsort(tgt, kind="stable")
    ssrc, stgt = src[order], tgt[order]
    deg_full = np.bincount(tgt, minlength=N).astype(np.float32)
    n_nt = (N + 127) // 128
    NPAD = n_nt * 128             # h_tab rows incl. zero padding

    blk_bounds = []
    for c in range(N_CORES):
        bounds = [c * NPC + bb * 128 for bb in range(NBLK)] + [(c + 1) * NPC]
        blk_bounds.append(np.searchsorted(stgt, bounds))
    cnt = np.array([[blk_bounds[c][bb + 1] - blk_bounds[c][bb]
                     for bb in range(NBLK)] for c in range(N_CORES)])
    # edge tiles per block (uniform across cores)
    n_tiles_blk = np.maximum(1, (cnt.max(axis=0) + 127) // 128)
    NT = int(n_tiles_blk.sum())
    t_ofs_blk = np.concatenate([[0], np.cumsum(n_tiles_blk)]).astype(int)

    # Per-core slot arrays; tile t slot p = slot index t*128+p of the block.
    srcs_all = np.zeros((N_CORES, 128, NT), dtype=np.int64)
    rowid_np = np.full((N_CORES, 128, NT), -1.0, dtype=np.float32)
    for c in range(N_CORES):
        for bb in range(NBLK):
            lo, hi = blk_bounds[c][bb], blk_bounds[c][bb + 1]
            nslot = hi - lo
            base_node = c * NPC + bb * 128
            t0 = int(t_ofs_blk[bb])
            net = int(n_tiles_blk[bb])
            ne = net * 128
            if nslot > 0:
                o2 = np.argsort(ssrc[lo:hi], kind="stable")
                s_blk = ssrc[lo:hi][o2]
                pad_val = int(s_blk[-1])
                fl_s = np.full(ne, pad_val, dtype=np.int64)
                fl_r = np.full(ne, -1.0, dtype=np.float32)
                fl_s[:nslot] = s_blk
                fl_r[:nslot] = (stgt[lo:hi][o2] - base_node).astype(np.float32)
                srcs_all[c, :, t0:t0 + net] = fl_s.reshape(net, 128).T
                rowid_np[c, :, t0:t0 + net] = fl_r.reshape(net, 128).T
            # else: pad filled below from other cores
    for bb in range(NBLK):
        t0 = int(t_ofs_blk[bb])
        net = int(n_tiles_blk[bb])
        nonempty = [c for c in range(N_CORES) if cnt[c][bb] > 0]
        if nonempty and len(nonempty) < N_CORES:
            ref = int(srcs_all[nonempty[0], 0, t0])
            for c in range(N_CORES):
                if cnt[c][bb] == 0:
                    srcs_all[c, :, t0:t0 + net] = ref

    # Gather groups: consecutive tiles of one block, <= GRP tiles,
    # cross-core index span <= SPAN.
    groups = []          # (block, tile_lo, n_tiles, base)
    for bb in range(NBLK):
        net = int(n_tiles_blk[bb])
        t0 = int(t_ofs_blk[bb])
        t = 0
        while t < net:
            best = 1
            for w in range(2, min(GRP, net - t) + 1):
                sl = srcs_all[:, :, t0 + t:t0 + t + w]
                if sl.max() - sl.min() > SPAN:
                    break
                best = w
            sl = srcs_all[:, :, t0 + t:t0 + t + best]
            assert sl.max() - sl.min() <= 32000, "single tile span too large"
            groups.append((bb, t, best, int(sl.min())))
            t += best
    groups_by_block = [[] for _ in range(NBLK)]
    for gi, g in enumerate(groups):
        groups_by_block[g[0]].append((gi,) + g[1:])

    g_cols = [(g[2] * 128) // 16 for g in groups]
    g_col_ofs = np.concatenate([[0], np.cumsum(g_cols)]).astype(int)
    IDXC = int(g_col_ofs[-1])
    idx16_np = np.zeros((N_CORES, 128, IDXC), dtype=np.int16)
    for c in range(N_CORES):
        for gi, (bb, tl, w, base) in enumerate(groups):
            t0 = int(t_ofs_blk[bb]) + tl
            rel = (srcs_all[c, :, t0:t0 + w] - base).astype(np.int16)  # [128, w]
            flat = rel.T.reshape(-1)                 # slot order t*128+p
            wrapped = flat.reshape(-1, 16).T         # [16, w*128/16]
            idx16_np[c, :, g_col_ofs[gi]:g_col_ofs[gi + 1]] = np.tile(wrapped, (8, 1))

    # Both one-hot orientations, host-built and uploaded as fp8 (0/1 exact):
    #   oh [slot_p, t*128+tgt]  — lhsT of the accumulation matmul
    #   ohT[tgt_p,  t*128+slot] — lhsT of the s1-per-slot matmul
    f8np = ml_dtypes.float8_e4m3fn
    oh_np = np.zeros((N_CORES, 128, NT * 128), dtype=f8np)
    ohT_np = np.zeros((N_CORES, 128, NT * 128), dtype=f8np)
    for c in range(N_CORES):
        rid = rowid_np[c].astype(np.int64)          # [slot, t], -1 = invalid
        for t in range(NT):
            valid = rid[:, t] >= 0
            slots = np.nonzero(valid)[0]
            tgts = rid[slots, t]
            oh_np[c, slots, t * 128 + tgts] = 1.0
            ohT_np[c, tgts, t * 128 + slots] = 1.0

    # Per-core own-node inputs for phase 1b (the core's target rows)
    OWNW = NBLK * 128
    nft_own_np = np.zeros((N_CORES, 128, OWNW), dtype=bfnp)
    deg_own_np = np.zeros((N_CORES, 128, NBLK), dtype=bfnp)
    for c in range(N_CORES):
        nft_own_np[c, :, :NPC] = NFT[:, c * NPC:(c + 1) * NPC]
        dcol = np.zeros(OWNW, dtype=np.float32)
        dcol[:NPC] = deg_full[c * NPC:(c + 1) * NPC]
        deg_own_np[c] = dcol.reshape(NBLK, 128).T.astype(bfnp)

    # ---------------- build the SPMD program ----------------
    # 64 KiB SWDGE scratch: a 1024-idx gather emits 1024 m2s+s2m descriptor
    # pairs; smaller rings make every big gather spin in await_space until
    # the previous gather on its queue fully drains.
    nc = bacc.Bacc("TRN2", target_bir_lowering=False, debug=False,
                   num_devices=N_CORES, num_swdge_queues=4,
                   dynamic_dma_scratch_size=49152)

    f8 = mybir.dt.float8e4
    nft_d = nc.dram_tensor("nft", [128, NPAD], bf16, kind="ExternalInput").ap()
    wt_d = nc.dram_tensor("wt", [128, HF], bf16, kind="ExternalInput").ap()
    m12_d = nc.dram_tensor("m12", [128, 2 * H], bf16, kind="ExternalInput").ap()
    brep_d = nc.dram_tensor("brep", [128, HF + 2 * H], fp32, kind="ExternalInput").ap()
    idx16_d = nc.dram_tensor("idx16", [128, IDXC], i16, kind="ExternalInput").ap()
    oh_d = nc.dram_tensor("ohp", [128, NT * 128], f8, kind="ExternalInput").ap()
    oht_d = nc.dram_tensor("ohtp", [128, NT * 128], f8, kind="ExternalInput").ap()
    nfto_d = nc.dram_tensor("nft_own", [128, OWNW], bf16, kind="ExternalInput").ap()
    dego_d = nc.dram_tensor("deg_own", [128, NBLK], bf16, kind="ExternalInput").ap()

    h_tab = nc.dram_tensor("h_tab", [NPAD, ROW], f8).ap()
    out_d = nc.dram_tensor("out", [NPC, HF], fp32, kind="ExternalOutput").ap()

    CW = HF + H       # 136: [Msg | ex] combo width
    SW = HF + 2 * H   # 144: phase-1 psum width
    OSW = SW + 1      # own-row width incl. deg
    MAXT = int(n_tiles_blk.max())

    with tile.TileContext(nc) as tc:
        with ExitStack() as ctx:
            cpool = ctx.enter_context(tc.tile_pool(name="consts", bufs=1))
            p1 = ctx.enter_context(tc.tile_pool(name="p1", bufs=3))
            p1ps = ctx.enter_context(tc.tile_pool(name="p1ps", bufs=2, space="PSUM"))
            gpool = ctx.enter_context(tc.tile_pool(name="gbuf", bufs=8))
            gp = ctx.enter_context(tc.tile_pool(name="gather", bufs=3))
            mp = ctx.enter_context(tc.tile_pool(name="meta", bufs=4))
            ps_acc = ctx.enter_context(tc.tile_pool(name="ps_acc", bufs=2, space="PSUM"))
            ps_z = ctx.enter_context(tc.tile_pool(name="ps_z", bufs=3, space="PSUM"))
            fin = ctx.enter_context(tc.tile_pool(name="fin", bufs=4))

            nc.gpsimd.load_library(library_config.mlp)

            wt_sb = cpool.tile([128, HF], bf16)
            nc.sync.dma_start(wt_sb[:], wt_d[:])
            m12_sb = cpool.tile([128, 2 * H], bf16)
            nc.sync.dma_start(m12_sb[:], m12_d[:])
            brep_sb = cpool.tile([128, SW], fp32)
            nc.sync.dma_start(brep_sb[:], brep_d[:])
            idx_sb = cpool.tile([128, IDXC], i16)
            nc.sync.dma_start(idx_sb[:], idx16_d[:])
            dego_sb = cpool.tile([128, NBLK], bf16)
            nc.sync.dma_start(dego_sb[:], dego_d[:])
            # SBUF-resident own tables: h pre-scaled by degree (bf16) and
            # the target-side attention score s1 (fp32)
            own_h = cpool.tile([128, NBLK, HF], bf16)
            own_s1 = cpool.tile([128, NBLK, H], fp32)

            b_is_zero = not np.any(b_ext)

            # ---------- phase 1b: own rows -> resident SBUF table ----------
            for ob0 in range(0, NBLK, 2):
                nk = min(2, NBLK - ob0)
                nfo = p1.tile([128, 256], bf16, tag="nfo")
                nc.sync.dma_start(nfo[:, :nk * 128],
                                  nfto_d[:, ob0 * 128:ob0 * 128 + nk * 128])
                ps = p1ps.tile([128, 2, SW], fp32, space="PSUM", tag="p1ps")
                for k in range(nk):
                    nc.tensor.matmul(ps[:, k, 0:HF],
                                     lhsT=nfo[:, k * 128:(k + 1) * 128],
                                     rhs=wt_sb[:], start=True, stop=True)
                    nc.tensor.matmul(ps[:, k, HF:SW],
                                     lhsT=nfo[:, k * 128:(k + 1) * 128],
                                     rhs=m12_sb[:], start=True, stop=True)
                if not b_is_zero:
                    nc.vector.tensor_tensor(
                        out=ps[:, :nk, :], in0=ps[:, :nk, :],
                        in1=brep_sb[:].unsqueeze(1).broadcast_to([128, nk, SW]),
                        op=OP.add)
                nc.vector.tensor_copy(own_s1[:, ob0:ob0 + nk, :],
                                      ps[:, :nk, SW - H:SW])
                # h pre-scaled by degree: the tail skip term is a plain add
                for k in range(nk):
                    nc.vector.tensor_tensor(
                        out=own_h[:, ob0 + k, :],
                        in0=ps[:, k, 0:HF],
                        in1=dego_sb[:, ob0 + k:ob0 + k + 1].broadcast_to(
                            [128, HF]),
                        op=OP.mult)

            # ---------- phase 1a: full h table (replicated) ----------
            # One 1024-node chunk per iteration: 8 matmul pairs into four
            # PSUM tiles, PSUM->SBUF casts alternating DVE/ACT, one write.
            CH = 1024
            for j0 in range(0, NPAD, CH):
                w = min(CH, NPAD - j0)
                nfc = p1.tile([128, CH], bf16, tag="nfc")
                nc.sync.dma_start(nfc[:, :w], nft_d[:, j0:j0 + w])
                nkc = (w + 127) // 128
                hrow = p1.tile([128, 8, ROW], f8, tag="hrow")
                for k0 in range(0, w, 256):
                    kw2 = min(256, w - k0)
                    nk = (kw2 + 127) // 128
                    ps = p1ps.tile([128, 2, SW], fp32, space="PSUM", tag="p1ps")
                    for k in range(nk):
                        kk = k0 + k * 128
                        nc.tensor.matmul(ps[:, k, 0:HF],
                                         lhsT=nfc[:, kk:kk + 128],
                                         rhs=wt_sb[:], start=True, stop=True)
                        nc.tensor.matmul(ps[:, k, HF:SW],
                                         lhsT=nfc[:, kk:kk + 128],
                                         rhs=m12_sb[:], start=True, stop=True)
                    if not b_is_zero:
                        nc.vector.tensor_tensor(
                            out=ps[:, :nk, :], in0=ps[:, :nk, :],
                            in1=brep_sb[:].unsqueeze(1).broadcast_to([128, nk, SW]),
                            op=OP.add)
                    ko = k0 // 128
                    # h as fp8, s2 as bf16 packed behind it (bitcast view);
                    # alternate engines so neither sequencer serializes
                    if (k0 // 256) % 2 == 0:
                        nc.vector.tensor_copy(hrow[:, ko:ko + nk, 0:HF],
                                              ps[:, :nk, 0:HF])
                        nc.vector.tensor_copy(
                            hrow[:, ko:ko + nk, HF:HF + 2 * H].bitcast(bf16),
                            ps[:, :nk, HF:HF + H])
                    else:
                        nc.scalar.copy(hrow[:, ko:ko + nk, 0:HF],
                                       ps[:, :nk, 0:HF])
                        nc.scalar.copy(
                            hrow[:, ko:ko + nk, HF:HF + 2 * H].bitcast(bf16),
                            ps[:, :nk, HF:HF + H])
                nc.scalar.dma_start(
                    h_tab[j0:j0 + w, :].rearrange("(k p) r -> p k r", k=nkc),
                    hrow[:, :nkc, :])

            # ---------- phase 2: edge processing (software-pipelined) ----------
            blk_state = {}

            def stage_fetch(bb, qn0):
                """A: dma_gather the block's edge rows; B: build both one-hot
                orientations; C: s1-per-slot matmuls."""
                net = int(n_tiles_blk[bb])
                t0 = int(t_ofs_blk[bb])
                qn = qn0

                G = gpool.tile([128, MAXT, ROW], f8, tag="G")
                for gi, tl, wdt, base in groups_by_block[bb]:
                    nc.gpsimd.dma_gather(
                        out_ap=G[:, tl:tl + wdt, :],
                        in_ap=h_tab[base:, :],
                        idxs_ap=idx_sb[:, g_col_ofs[gi]:g_col_ofs[gi + 1]],
                        num_idxs=wdt * 128, num_idxs_reg=wdt * 128,
                        elem_size=ROW, queue_num=qn % 4)
                    qn += 1

                # both one-hot orientations stream in as fp8 matmul weights
                oh = gp.tile([128, MAXT, 128], f8, tag="oh")
                nc.sync.dma_start(oh[:, :net, :],
                                  oh_d[:, t0 * 128:(t0 + net) * 128])
                ohT = gp.tile([128, MAXT, 128], f8, tag="ohT")
                nc.sync.dma_start(ohT[:, :net, :],
                                  oht_d[:, t0 * 128:(t0 + net) * 128])

                # s1 of the block's targets (bf16 rhs, cast on ACT), then the
                # per-slot score z = s1[tgt] + s2[src] built entirely in PSUM:
                # ohT matmul adds s1e, identity matmul accumulates the
                # gathered s2 -- no DVE involvement
                s1bf = mp.tile([128, H], bf16, tag="s1bf")
                nc.scalar.copy(s1bf[:], own_s1[:, bb, :])
                zps = ps_z.tile([128, MAXT, H], fp32, space="PSUM", tag="zps")
                for t in range(net):
                    nc.tensor.matmul(zps[:, t, :], lhsT=ohT[:, t, :],
                                     rhs=s1bf[:], start=True, stop=True)

                blk_state[bb] = (G, oh, zps, qn0)
                return qn

            def stage_front(bb):
                """D1: scores z -> lrelu -> exp -> expanded weights."""
                net = int(n_tiles_blk[bb])
                G, oh, zps, _ = blk_state[bb]

                me = gp.tile([128, MAXT, CW], bf16, tag="me")
                # upcast the gathered fp8 h to bf16 on ACT (independent of
                # the score chain) so the big DVE multiply stays at bf16 rate
                gh16 = gp.tile([128, MAXT, HF], bf16, tag="gh16")
                nc.scalar.copy(gh16[:, :net, :], G[:, :net, 0:HF])
                z_sb = mp.tile([128, MAXT, H], fp32, tag="z_sb")
                nc.vector.tensor_tensor(
                    out=z_sb[:, :net, :], in0=zps[:, :net, :],
                    in1=G[:, :net, HF:HF + 2 * H].bitcast(bf16), op=OP.add)
                ext = mp.tile([128, MAXT, H], bf16, tag="ext")
                nc.vector.scalar_tensor_tensor(
                    out=ext[:, :net, :], in0=z_sb[:, :net, :], scalar=slope,
                    in1=z_sb[:, :net, :], op0=OP.mult, op1=OP.max)
                # exp lands directly in the combo tile's ex columns
                nc.scalar.activation(me[:, :net, HF:CW], ext[:, :net, :], AF.Exp)
                # expand ex across F_OUT on ACT so the big DVE multiply is
                # contiguous x contiguous (2x bf16 rate)
                ex128 = gp.tile([128, MAXT, H, F_OUT], bf16, tag="ex128")
                nc.scalar.copy(
                    ex128[:, :net, :, :],
                    me[:, :net, HF:CW].unsqueeze(3).broadcast_to(
                        [128, net, H, F_OUT]))
                blk_state[bb] = (gh16, oh, me, ex128)

            def stage_back(bb):
                """D2: weighted messages and the accumulation matmuls."""
                net = int(n_tiles_blk[bb])
                gh16, oh, me, ex128 = blk_state[bb]

                acc = ps_acc.tile([128, CW], fp32, space="PSUM", tag="acc")
                nc.vector.tensor_tensor(
                    out=me[:, 0:net, 0:HF], in0=gh16[:, 0:net, :],
                    in1=ex128[:, 0:net, :, :], op=OP.mult)
                for t in range(net):
                    nc.tensor.matmul(acc[:, :], lhsT=oh[:, t, :],
                                     rhs=me[:, t, :],
                                     start=(t == 0), stop=(t == net - 1))
                blk_state[bb] = acc

            # Tails are batched BT blocks at a time: each block's PSUM is
            # drained to SBUF on ACT right away (freeing the acc bank), and
            # the division/skip/ELU chain runs once per batch so the tail
            # leaves the per-block critical cycle.
            BT = 4
            batch_accs = {}

            def stage_drain(bb):
                b0 = (bb // BT) * BT
                if bb == b0:
                    accs_t = fin.tile([128, BT, CW], fp32, tag="accs", bufs=3)
                    batch_accs[b0] = accs_t
                acc = blk_state.pop(bb)
                nc.scalar.copy(batch_accs[b0][:, bb - b0, :], acc[:, :])

            def stage_tail_batch(b0):
                nb = min(BT, NBLK - b0)
                accs = batch_accs.pop(b0)
                rec = fin.tile([128, BT, H], fp32, tag="rec", bufs=2)
                nc.vector.tensor_scalar_add(out=rec[:, :nb, :],
                                            in0=accs[:, :nb, HF:CW],
                                            scalar1=1e-30)
                nc.vector.reciprocal(rec[:, :nb, :], rec[:, :nb, :])
                nrm = fin.tile([128, BT, HF], fp32, tag="nrm", bufs=2)
                nc.vector.tensor_tensor(
                    out=nrm[:, :nb, :], in0=accs[:, :nb, 0:HF],
                    in1=rec[:, :nb, :].unsqueeze(3).broadcast_to(
                        [128, nb, H, F_OUT]),
                    op=OP.mult)
                # += deg * h_own (h_own pre-scaled by deg in phase 1b)
                nc.vector.tensor_tensor(out=nrm[:, :nb, :], in0=nrm[:, :nb, :],
                                        in1=own_h[:, b0:b0 + nb, :], op=OP.add)
                # ELU = max(x, exp(min(x,0)) - 1)
                neg = fin.tile([128, BT, HF], fp32, tag="neg", bufs=2)
                nc.vector.tensor_scalar_min(out=neg[:, :nb, :],
                                            in0=nrm[:, :nb, :], scalar1=0.0)
                nc.scalar.activation(neg[:, :nb, :], neg[:, :nb, :], AF.Exp)
                res = fin.tile([128, BT, HF], fp32, tag="res", bufs=2)
                nc.vector.scalar_tensor_tensor(
                    out=res[:, :nb, :], in0=neg[:, :nb, :], scalar=-1.0,
                    in1=nrm[:, :nb, :], op0=OP.add, op1=OP.max)
                base_row = b0 * 128
                nrows = min(BT * 128, NPC - base_row)
                nfull = nrows // 128
                if nfull > 0:
                    nc.scalar.dma_start(
                        out_d[base_row:base_row + nfull * 128, :].rearrange(
                            "(k p) r -> p k r", k=nfull),
                        res[:, :nfull, :])
                rem = nrows - nfull * 128
                if rem > 0:
                    nc.scalar.dma_start(
                        out_d[base_row + nfull * 128:base_row + nrows, :],
                        res[:rem, nfull, :])

            # tails run TS blocks behind the drains so their first DVE op
            # never blocks the queue waiting for the batch to fill
            TS = 3
            qn = 0
            for i in range(NBLK + 4 + TS):
                if i < NBLK:
                    qn = stage_fetch(i, qn)
                if 2 <= i <= NBLK + 1:
                    stage_front(i - 2)
                if 3 <= i <= NBLK + 2:
                    stage_back(i - 3)
                if 4 <= i <= NBLK + 3:
                    stage_drain(i - 4)
                if i >= 4 + TS:
                    bb = i - 4 - TS
                    if bb % BT == BT - 1 or bb == NBLK - 1:
                        stage_tail_batch((bb // BT) * BT)

    nc.compile()

    in_maps = []
    for c in range(N_CORES):
        in_maps.append({
            "nft": _pad_cols(NFT, NPAD), "wt": WT.astype(bfnp),
            "m12": M12.astype(bfnp), "brep": b_rep,
            "idx16": idx16_np[c], "ohp": oh_np[c], "ohtp": ohT_np[c],
            "nft_own": nft_own_np[c], "deg_own": deg_own_np[c],
        })
    import os
    trace = bool(os.environ.get("GAT_TRACE"))
    if trace:
        _install_ntff_hook()
    res = run_bass_kernel_spmd(nc, in_maps, list(range(N_CORES)), trace=trace)
    global _last_results
    _last_results = res
    out = np.concatenate([res.results[c]["out"] for c in range(N_CORES)], axis=0)
    return out


def _pad_cols(arr, cols):
    if arr.shape[1] == cols:
        return arr
    out = np.zeros((arr.shape[0], cols), dtype=arr.dtype)
    out[:, :arr.shape[1]] = arr
    return out


# revision 61
# speedup vs baseline: 1.1553x; 1.1553x over previous
"""Multi-head GAT layer on 8 Trainium2 NeuronCores (Bass/Tile SPMD kernel).

Strategy (edge-parallel, target-sharded):
  - Edges sorted by target, sharded across 8 cores by contiguous target
    ranges (N/8 nodes each): softmax + aggregation are core-local.
  - Phase 1a (replicated on every core): one bf16 PE pass over the node
    features builds a per-node table row [ h (128) | s2 (8) | s1 (8) ]
    (bf16, 512B rows) where h = NF @ W.T + b and s1/s2 are the per-node
    attention scores h . a1 / h . a2 (fused into the same matmul via
    W.T @ A12).
  - Phase 1b (per-core data, same program): the core's own 6250 target
    rows are recomputed into a resident SBUF table (fp32, including
    degree) so phase 2 needs no self-row gather at all.
  - Phase 2, software-pipelined per 128-target block:
      A: edge slots (padded to 128-slot tiles, sorted by src) fetched
         with dma_gather (int16 indices + static per-group base, 4 SWDGE
         queues, 64KB descriptor rings);
      B: slot->target one-hots built on DVE: oh from the resident rowid
         table, ohT from a host-uploaded free-axis rowid pattern (int8)
         so no PE transposes are needed;
      C: s1-per-slot via small PE matmuls against ohT;
      D: scores z = s1e+s2, ex = exp(lrelu(z)) (DVE+ACT), ex expanded
         across F_OUT on ACT so the big DVE multiply runs contiguous;
         a single PE matmul per tile accumulates [Msg | ex] into PSUM;
      E: tail = softmax division, skip term from the SBUF own-table,
         ELU as max(x, exp(min(x,0))-1), contiguous output write.
    Stages are emitted skewed (A/B/C for block i, D for i-1, E for i-2)
    so each in-order engine queue interleaves independent blocks.
"""

import numpy as np

N_CORES = 8
_last_results = None  # BassKernelResults of the most recent run (for harnesses)


def _install_ntff_hook():
    """Register the axon NTFF profiling hook if the image lacks antenv.axon_hooks."""
    import sys, types
    try:
        from antenv.axon_hooks import get_axon_ntff_profile_hook  # noqa: F401
        return
    except ImportError:
        pass
    try:
        mod = types.ModuleType("antenv.axon_hooks")
        holder = [None]
        mod.set_axon_ntff_profile_hook = lambda h: holder.__setitem__(0, h)
        mod.get_axon_ntff_profile_hook = lambda: holder[0]
        sys.modules["antenv.axon_hooks"] = mod
        from trn_agent_boot.trn_boot import _ntff_profile_via_ctypes
        mod.set_axon_ntff_profile_hook(
            _ntff_profile_via_ctypes("/opt/axon/libaxon_pjrt.so"))
    except Exception:
        sys.modules.pop("antenv.axon_hooks", None)


def kernel(node_features, edge_index, W, b, a):
    return gat_multicore(
        np.asarray(node_features, dtype=np.float32),
        np.asarray(edge_index, dtype=np.int32),
        np.asarray(W, dtype=np.float32),
        np.asarray(b, dtype=np.float32),
        np.asarray(a, dtype=np.float32),
    )


def gat_multicore(nf, ei, W, b, a, slope=0.2):
    import sys
    if "/opt/trn_rl_repo" not in sys.path:
        sys.path.insert(0, "/opt/trn_rl_repo")
    import ml_dtypes
    import concourse.bacc as bacc
    import concourse.tile as tile
    import concourse.mybir as mybir
    from concourse import library_config
    from concourse.bass_utils import run_bass_kernel_spmd
    from contextlib import ExitStack

    fp32 = mybir.dt.float32
    bf16 = mybir.dt.bfloat16
    i16 = mybir.dt.int16
    AF = mybir.ActivationFunctionType
    OP = mybir.AluOpType
    bfnp = ml_dtypes.bfloat16

    N, F_IN = nf.shape
    E = ei.shape[1]
    HF = W.shape[0]               # H * F_OUT
    F_OUT = a.shape[0] // 2
    H = HF // F_OUT
    assert F_IN == 128 and HF == 128, "kernel assumes 128 in/out features"
    assert N % N_CORES == 0
    NPC = N // N_CORES            # targets per core
    NBLK = (NPC + 127) // 128     # 128-target blocks per core
    GRP = 8                       # max tiles per gather group
    ROW = 256                     # bf16 elements per table row (512 B)
    SPAN = 30000                  # max int16 index span per gather group

    # ---------------- host prep: weights ----------------
    WT = np.ascontiguousarray(W.T)                       # [F_IN, HF]
    # A12 column order: [s2 (a2) | s1 (a1)] to match the table row layout
    A12 = np.zeros((HF, 2 * H), dtype=np.float32)
    for hd in range(H):
        A12[hd * F_OUT:(hd + 1) * F_OUT, hd] = a[F_OUT:]        # s2
        A12[hd * F_OUT:(hd + 1) * F_OUT, H + hd] = a[:F_OUT]    # s1
    M12 = (WT @ A12).astype(np.float32)                  # [F_IN, 2H]
    b12 = (b @ A12).astype(np.float32)                   # [2H]
    b_ext = np.concatenate([b, b12]).astype(np.float32)  # [144]
    b_rep = np.broadcast_to(b_ext, (128, HF + 2 * H)).copy()
    NFT = np.ascontiguousarray(nf.T).astype(bfnp)        # [F_IN, N] bf16

    # ---------------- host prep: graph structure ----------------
    src, tgt = ei[0].astype(np.int64), ei[1].astype(np.int64)
    order = np.argsort(tgt, kind="stable")
    ssrc, stgt = src[order], tgt[order]
    deg_full = np.bincount(tgt, minlength=N).astype(np.float32)
    n_nt = (N + 127) // 128
    NPAD = n_nt * 128             # h_tab rows incl. zero padding

    blk_bounds = []
    for c in range(N_CORES):
        bounds = [c * NPC + bb * 128 for bb in range(NBLK)] + [(c + 1) * NPC]
        blk_bounds.append(np.searchsorted(stgt, bounds))
    cnt = np.array([[blk_bounds[c][bb + 1] - blk_bounds[c][bb]
                     for bb in range(NBLK)] for c in range(N_CORES)])
    # edge tiles per block (uniform across cores)
    n_tiles_blk = np.maximum(1, (cnt.max(axis=0) + 127) // 128)
    NT = int(n_tiles_blk.sum())
    t_ofs_blk = np.concatenate([[0], np.cumsum(n_tiles_blk)]).astype(int)

    # Per-core slot arrays; tile t slot p = slot index t*128+p of the block.
    srcs_all = np.zeros((N_CORES, 128, NT), dtype=np.int64)
    rowid_np = np.full((N_CORES, 128, NT), -1.0, dtype=np.float32)
    for c in range(N_CORES):
        for bb in range(NBLK):
            lo, hi = blk_bounds[c][bb], blk_bounds[c][bb + 1]
            nslot = hi - lo
            base_node = c * NPC + bb * 128
            t0 = int(t_ofs_blk[bb])
            net = int(n_tiles_blk[bb])
            ne = net * 128
            if nslot > 0:
                o2 = np.argsort(ssrc[lo:hi], kind="stable")
                s_blk = ssrc[lo:hi][o2]
                pad_val = int(s_blk[-1])
                fl_s = np.full(ne, pad_val, dtype=np.int64)
                fl_r = np.full(ne, -1.0, dtype=np.float32)
                fl_s[:nslot] = s_blk
                fl_r[:nslot] = (stgt[lo:hi][o2] - base_node).astype(np.float32)
                srcs_all[c, :, t0:t0 + net] = fl_s.reshape(net, 128).T
                rowid_np[c, :, t0:t0 + net] = fl_r.reshape(net, 128).T
            # else: pad filled below from other cores
    for bb in range(NBLK):
        t0 = int(t_ofs_blk[bb])
        net = int(n_tiles_blk[bb])
        nonempty = [c for c in range(N_CORES) if cnt[c][bb] > 0]
        if nonempty and len(nonempty) < N_CORES:
            ref = int(srcs_all[nonempty[0], 0, t0])
            for c in range(N_CORES):
                if cnt[c][bb] == 0:
                    srcs_all[c, :, t0:t0 + net] = ref

    # Gather groups: consecutive tiles of one block, <= GRP tiles,
    # cross-core index span <= SPAN.
    groups = []          # (block, tile_lo, n_tiles, base)
    for bb in range(NBLK):
        net = int(n_tiles_blk[bb])
        t0 = int(t_ofs_blk[bb])
        t = 0
        while t < net:
            best = 1
            for w in range(2, min(GRP, net - t) + 1):
                sl = srcs_all[:, :, t0 + t:t0 + t + w]
                if sl.max() - sl.min() > SPAN:
                    break
                best = w
            sl = srcs_all[:, :, t0 + t:t0 + t + best]
            assert sl.max() - sl.min() <= 32000, "single tile span too large"
            groups.append((bb, t, best, int(sl.min())))
            t += best
    groups_by_block = [[] for _ in range(NBLK)]
    for gi, g in enumerate(groups):
        groups_by_block[g[0]].append((gi,) + g[1:])

    g_cols = [(g[2] * 128) // 16 for g in groups]
    g_col_ofs = np.concatenate([[0], np.cumsum(g_cols)]).astype(int)
    IDXC = int(g_col_ofs[-1])
    idx16_np = np.zeros((N_CORES, 128, IDXC), dtype=np.int16)
    for c in range(N_CORES):
        for gi, (bb, tl, w, base) in enumerate(groups):
            t0 = int(t_ofs_blk[bb]) + tl
            rel = (srcs_all[c, :, t0:t0 + w] - base).astype(np.int16)  # [128, w]
            flat = rel.T.reshape(-1)                 # slot order t*128+p
            wrapped = flat.reshape(-1, 16).T         # [16, w*128/16]
            idx16_np[c, :, g_col_ofs[gi]:g_col_ofs[gi + 1]] = np.tile(wrapped, (8, 1))

    # Both one-hot orientations, host-built and uploaded as fp8 (0/1 exact):
    #   oh [slot_p, t*128+tgt]  — lhsT of the accumulation matmul
    #   ohT[tgt_p,  t*128+slot] — lhsT of the s1-per-slot matmul
    f8np = ml_dtypes.float8_e4m3fn
    oh_np = np.zeros((N_CORES, 128, NT * 128), dtype=f8np)
    ohT_np = np.zeros((N_CORES, 128, NT * 128), dtype=f8np)
    for c in range(N_CORES):
        rid = rowid_np[c].astype(np.int64)          # [slot, t], -1 = invalid
        for t in range(NT):
            valid = rid[:, t] >= 0
            slots = np.nonzero(valid)[0]
            tgts = rid[slots, t]
            oh_np[c, slots, t * 128 + tgts] = 1.0
            ohT_np[c, tgts, t * 128 + slots] = 1.0

    # Per-core own-node inputs for phase 1b (the core's target rows)
    OWNW = NBLK * 128
    nft_own_np = np.zeros((N_CORES, 128, OWNW), dtype=bfnp)
    deg_own_np = np.zeros((N_CORES, 128, NBLK), dtype=bfnp)
    for c in range(N_CORES):
        nft_own_np[c, :, :NPC] = NFT[:, c * NPC:(c + 1) * NPC]
        dcol = np.zeros(OWNW, dtype=np.float32)
        dcol[:NPC] = deg_full[c * NPC:(c + 1) * NPC]
        deg_own_np[c] = dcol.reshape(NBLK, 128).T.astype(bfnp)

    # ---------------- build the SPMD program ----------------
    # 64 KiB SWDGE scratch: a 1024-idx gather emits 1024 m2s+s2m descriptor
    # pairs; smaller rings make every big gather spin in await_space until
    # the previous gather on its queue fully drains.
    nc = bacc.Bacc("TRN2", target_bir_lowering=False, debug=False,
                   num_devices=N_CORES, num_swdge_queues=4,
                   dynamic_dma_scratch_size=36864)

    f8 = mybir.dt.float8e4
    nft_d = nc.dram_tensor("nft", [128, NPAD], bf16, kind="ExternalInput").ap()
    wt_d = nc.dram_tensor("wt", [128, HF], bf16, kind="ExternalInput").ap()
    m12_d = nc.dram_tensor("m12", [128, 2 * H], bf16, kind="ExternalInput").ap()
    brep_d = nc.dram_tensor("brep", [128, HF + 2 * H], fp32, kind="ExternalInput").ap()
    idx16_d = nc.dram_tensor("idx16", [128, IDXC], i16, kind="ExternalInput").ap()
    oh_d = nc.dram_tensor("ohp", [128, NT * 128], f8, kind="ExternalInput").ap()
    oht_d = nc.dram_tensor("ohtp", [128, NT * 128], f8, kind="ExternalInput").ap()
    nfto_d = nc.dram_tensor("nft_own", [128, OWNW], bf16, kind="ExternalInput").ap()
    dego_d = nc.dram_tensor("deg_own", [128, NBLK], bf16, kind="ExternalInput").ap()

    h_tab = nc.dram_tensor("h_tab", [NPAD, ROW], bf16).ap()
    out_d = nc.dram_tensor("out", [NPC, HF], fp32, kind="ExternalOutput").ap()

    CW = HF + H       # 136: [Msg | ex] combo width
    SW = HF + 2 * H   # 144: phase-1 psum width
    OSW = SW + 1      # own-row width incl. deg
    MAXT = int(n_tiles_blk.max())

    with tile.TileContext(nc) as tc:
        with ExitStack() as ctx:
            cpool = ctx.enter_context(tc.tile_pool(name="consts", bufs=1))
            p1 = ctx.enter_context(tc.tile_pool(name="p1", bufs=3))
            p1ps = ctx.enter_context(tc.tile_pool(name="p1ps", bufs=2, space="PSUM"))
            gpool = ctx.enter_context(tc.tile_pool(name="gbuf", bufs=6))
            gp = ctx.enter_context(tc.tile_pool(name="gather", bufs=3))
            mp = ctx.enter_context(tc.tile_pool(name="meta", bufs=4))
            ps_acc = ctx.enter_context(tc.tile_pool(name="ps_acc", bufs=3, space="PSUM"))
            ps_z = ctx.enter_context(tc.tile_pool(name="ps_z", bufs=3, space="PSUM"))
            fin = ctx.enter_context(tc.tile_pool(name="fin", bufs=4))

            nc.gpsimd.load_library(library_config.mlp)

            wt_sb = cpool.tile([128, HF], bf16)
            nc.sync.dma_start(wt_sb[:], wt_d[:])
            m12_sb = cpool.tile([128, 2 * H], bf16)
            nc.sync.dma_start(m12_sb[:], m12_d[:])
            brep_sb = cpool.tile([128, SW], fp32)
            nc.sync.dma_start(brep_sb[:], brep_d[:])
            idx_sb = cpool.tile([128, IDXC], i16)
            nc.sync.dma_start(idx_sb[:], idx16_d[:])
            dego_sb = cpool.tile([128, NBLK], bf16)
            nc.sync.dma_start(dego_sb[:], dego_d[:])
            # SBUF-resident own tables: h pre-scaled by degree (bf16) and
            # the target-side attention score s1 (fp32)
            own_h = cpool.tile([128, NBLK, HF], bf16)
            own_s1 = cpool.tile([128, NBLK, H], fp32)

            b_is_zero = not np.any(b_ext)

            # ---------- phase 1b: own rows -> resident SBUF table ----------
            for ob0 in range(0, NBLK, 2):
                nk = min(2, NBLK - ob0)
                nfo = p1.tile([128, 256], bf16, tag="nfo")
                nc.sync.dma_start(nfo[:, :nk * 128],
                                  nfto_d[:, ob0 * 128:ob0 * 128 + nk * 128])
                ps = p1ps.tile([128, 2, SW], fp32, space="PSUM", tag="p1ps")
                for k in range(nk):
                    nc.tensor.matmul(ps[:, k, 0:HF],
                                     lhsT=nfo[:, k * 128:(k + 1) * 128],
                                     rhs=wt_sb[:], start=True, stop=True)
                    nc.tensor.matmul(ps[:, k, HF:SW],
                                     lhsT=nfo[:, k * 128:(k + 1) * 128],
                                     rhs=m12_sb[:], start=True, stop=True)
                if not b_is_zero:
                    nc.vector.tensor_tensor(
                        out=ps[:, :nk, :], in0=ps[:, :nk, :],
                        in1=brep_sb[:].unsqueeze(1).broadcast_to([128, nk, SW]),
                        op=OP.add)
                nc.vector.tensor_copy(own_s1[:, ob0:ob0 + nk, :],
                                      ps[:, :nk, SW - H:SW])
                # h pre-scaled by degree: the tail skip term is a plain add
                for k in range(nk):
                    nc.vector.tensor_tensor(
                        out=own_h[:, ob0 + k, :],
                        in0=ps[:, k, 0:HF],
                        in1=dego_sb[:, ob0 + k:ob0 + k + 1].broadcast_to(
                            [128, HF]),
                        op=OP.mult)

            # ---------- phase 1a: full h table (replicated) ----------
            # One 512-node chunk per iteration: 4 matmul pairs into two PSUM
            # tiles, PSUM->SBUF casts alternating DVE/ACT, one table write.
            CH = 512
            for j0 in range(0, NPAD, CH):
                w = min(CH, NPAD - j0)
                nfc = p1.tile([128, CH], bf16, tag="nfc")
                nc.sync.dma_start(nfc[:, :w], nft_d[:, j0:j0 + w])
                nkc = (w + 127) // 128
                hrow = p1.tile([128, 4, ROW], bf16, tag="hrow")
                for k0 in range(0, w, 256):
                    kw2 = min(256, w - k0)
                    nk = (kw2 + 127) // 128
                    ps = p1ps.tile([128, 2, SW], fp32, space="PSUM", tag="p1ps")
                    for k in range(nk):
                        kk = k0 + k * 128
                        nc.tensor.matmul(ps[:, k, 0:HF],
                                         lhsT=nfc[:, kk:kk + 128],
                                         rhs=wt_sb[:], start=True, stop=True)
                        nc.tensor.matmul(ps[:, k, HF:SW],
                                         lhsT=nfc[:, kk:kk + 128],
                                         rhs=m12_sb[:], start=True, stop=True)
                    ko = k0 // 128
                    if b_is_zero and k0 == 0:
                        nc.vector.tensor_copy(hrow[:, ko:ko + nk, 0:SW],
                                              ps[:, :nk, :])
                    elif b_is_zero:
                        nc.scalar.copy(hrow[:, ko:ko + nk, 0:SW], ps[:, :nk, :])
                    else:
                        nc.vector.tensor_tensor(
                            out=hrow[:, ko:ko + nk, 0:SW], in0=ps[:, :nk, :],
                            in1=brep_sb[:].unsqueeze(1).broadcast_to([128, nk, SW]),
                            op=OP.add)
                nc.scalar.dma_start(
                    h_tab[j0:j0 + w, :].rearrange("(k p) r -> p k r", k=nkc),
                    hrow[:, :nkc, :])

            # ---------- phase 2: edge processing (software-pipelined) ----------
            blk_state = {}

            def stage_fetch(bb, qn0):
                """A: dma_gather the block's edge rows; B: build both one-hot
                orientations; C: s1-per-slot matmuls."""
                net = int(n_tiles_blk[bb])
                t0 = int(t_ofs_blk[bb])
                qn = qn0

                G = gpool.tile([128, MAXT, ROW], bf16, tag="G")
                for gi, tl, wdt, base in groups_by_block[bb]:
                    nc.gpsimd.dma_gather(
                        out_ap=G[:, tl:tl + wdt, :],
                        in_ap=h_tab[base:, :],
                        idxs_ap=idx_sb[:, g_col_ofs[gi]:g_col_ofs[gi + 1]],
                        num_idxs=wdt * 128, num_idxs_reg=wdt * 128,
                        elem_size=ROW, queue_num=qn % 4)
                    qn += 1

                # both one-hot orientations stream in as fp8 matmul weights
                oh = gp.tile([128, MAXT, 128], f8, tag="oh")
                nc.sync.dma_start(oh[:, :net, :],
                                  oh_d[:, t0 * 128:(t0 + net) * 128])
                ohT = gp.tile([128, MAXT, 128], f8, tag="ohT")
                nc.sync.dma_start(ohT[:, :net, :],
                                  oht_d[:, t0 * 128:(t0 + net) * 128])

                # s1 of the block's targets (bf16 rhs, cast on ACT), then the
                # per-slot score z = s1[tgt] + s2[src] built entirely in PSUM:
                # ohT matmul adds s1e, identity matmul accumulates the
                # gathered s2 -- no DVE involvement
                s1bf = mp.tile([128, H], bf16, tag="s1bf")
                nc.scalar.copy(s1bf[:], own_s1[:, bb, :])
                zps = ps_z.tile([128, MAXT, H], fp32, space="PSUM", tag="zps")
                for t in range(net):
                    nc.tensor.matmul(zps[:, t, :], lhsT=ohT[:, t, :],
                                     rhs=s1bf[:], start=True, stop=True)

                blk_state[bb] = (G, oh, zps, qn0)
                return qn

            def stage_front(bb):
                """D1: scores z -> lrelu -> exp -> expanded weights."""
                net = int(n_tiles_blk[bb])
                G, oh, zps, _ = blk_state[bb]

                me = gp.tile([128, MAXT, CW], bf16, tag="me")
                z_sb = mp.tile([128, MAXT, H], fp32, tag="z_sb")
                nc.vector.tensor_tensor(out=z_sb[:, :net, :], in0=zps[:, :net, :],
                                        in1=G[:, :net, HF:HF + H], op=OP.add)
                ext = mp.tile([128, MAXT, H], bf16, tag="ext")
                nc.vector.scalar_tensor_tensor(
                    out=ext[:, :net, :], in0=z_sb[:, :net, :], scalar=slope,
                    in1=z_sb[:, :net, :], op0=OP.mult, op1=OP.max)
                # exp lands directly in the combo tile's ex columns
                nc.scalar.activation(me[:, :net, HF:CW], ext[:, :net, :], AF.Exp)
                # expand ex across F_OUT on ACT so the big DVE multiply is
                # contiguous x contiguous (2x bf16 rate)
                ex128 = gp.tile([128, MAXT, H, F_OUT], bf16, tag="ex128")
                nc.scalar.copy(
                    ex128[:, :net, :, :],
                    me[:, :net, HF:CW].unsqueeze(3).broadcast_to(
                        [128, net, H, F_OUT]))
                blk_state[bb] = (G, oh, me, ex128)

            def stage_back(bb):
                """D2: weighted messages and the accumulation matmuls."""
                net = int(n_tiles_blk[bb])
                G, oh, me, ex128 = blk_state[bb]

                acc = ps_acc.tile([128, CW], fp32, space="PSUM", tag="acc")
                nc.vector.tensor_tensor(
                    out=me[:, 0:net, 0:HF], in0=G[:, 0:net, 0:HF],
                    in1=ex128[:, 0:net, :, :], op=OP.mult)
                for t in range(net):
                    nc.tensor.matmul(acc[:, :], lhsT=oh[:, t, :],
                                     rhs=me[:, t, :],
                                     start=(t == 0), stop=(t == net - 1))
                blk_state[bb] = acc

            # Tails are batched BT blocks at a time: each block's PSUM is
            # drained to SBUF on ACT right away (freeing the acc bank), and
            # the division/skip/ELU chain runs once per batch so the tail
            # leaves the per-block critical cycle.
            BT = 6
            batch_accs = {}

            def stage_drain(bb):
                b0 = (bb // BT) * BT
                if bb == b0:
                    accs_t = fin.tile([128, BT, CW], fp32, tag="accs", bufs=3)
                    batch_accs[b0] = accs_t
                acc = blk_state.pop(bb)
                nc.scalar.copy(batch_accs[b0][:, bb - b0, :], acc[:, :])

            def stage_tail_batch(b0):
                nb = min(BT, NBLK - b0)
                accs = batch_accs.pop(b0)
                rec = fin.tile([128, BT, H], fp32, tag="rec", bufs=2)
                nc.vector.tensor_scalar_add(out=rec[:, :nb, :],
                                            in0=accs[:, :nb, HF:CW],
                                            scalar1=1e-30)
                nc.vector.reciprocal(rec[:, :nb, :], rec[:, :nb, :])
                nrm = fin.tile([128, BT, HF], fp32, tag="nrm", bufs=2)
                nc.vector.tensor_tensor(
                    out=nrm[:, :nb, :], in0=accs[:, :nb, 0:HF],
                    in1=rec[:, :nb, :].unsqueeze(3).broadcast_to(
                        [128, nb, H, F_OUT]),
                    op=OP.mult)
                # += deg * h_own (h_own pre-scaled by deg in phase 1b)
                nc.vector.tensor_tensor(out=nrm[:, :nb, :], in0=nrm[:, :nb, :],
                                        in1=own_h[:, b0:b0 + nb, :], op=OP.add)
                # ELU = max(x, exp(min(x,0)) - 1)
                neg = fin.tile([128, BT, HF], fp32, tag="neg", bufs=2)
                nc.vector.tensor_scalar_min(out=neg[:, :nb, :],
                                            in0=nrm[:, :nb, :], scalar1=0.0)
                nc.scalar.activation(neg[:, :nb, :], neg[:, :nb, :], AF.Exp)
                res = fin.tile([128, BT, HF], fp32, tag="res", bufs=2)
                nc.vector.scalar_tensor_tensor(
                    out=res[:, :nb, :], in0=neg[:, :nb, :], scalar=-1.0,
                    in1=nrm[:, :nb, :], op0=OP.add, op1=OP.max)
                base_row = b0 * 128
                nrows = min(BT * 128, NPC - base_row)
                nfull = nrows // 128
                if nfull > 0:
                    nc.scalar.dma_start(
                        out_d[base_row:base_row + nfull * 128, :].rearrange(
                            "(k p) r -> p k r", k=nfull),
                        res[:, :nfull, :])
                rem = nrows - nfull * 128
                if rem > 0:
                    nc.scalar.dma_start(
                        out_d[base_row + nfull * 128:base_row + nrows, :],
                        res[:rem, nfull, :])

            # tails run TS blocks behind the drains so their first DVE op
            # never blocks the queue waiting for the batch to fill
            TS = 3
            qn = 0
            for i in range(NBLK + 4 + TS):
                if i < NBLK:
                    qn = stage_fetch(i, qn)
                if 2 <= i <= NBLK + 1:
                    stage_front(i - 2)
                if 3 <= i <= NBLK + 2:
                    stage_back(i - 3)
                if 4 <= i <= NBLK + 3:
                    stage_drain(i - 4)
                if i >= 4 + TS:
                    bb = i - 4 - TS
                    if bb % BT == BT - 1 or bb == NBLK - 1:
                        stage_tail_batch((bb // BT) * BT)

    nc.compile()

    in_maps = []
    for c in range(N_CORES):
        in_maps.append({
            "nft": _pad_cols(NFT, NPAD), "wt": WT.astype(bfnp),
            "m12": M12.astype(bfnp), "brep": b_rep,
            "idx16": idx16_np[c], "ohp": oh_np[c], "ohtp": ohT_np[c],
            "nft_own": nft_own_np[c], "deg_own": deg_own_np[c],
        })
    import os
    trace = bool(os.environ.get("GAT_TRACE"))
    if trace:
        _install_ntff_hook()
    res = run_bass_kernel_spmd(nc, in_maps, list(range(N_CORES)), trace=trace)
    global _last_results
    _last_results = res
    out = np.concatenate([res.results[c]["out"] for c in range(N_CORES)], axis=0)
    return out


def _pad_cols(arr, cols):
    if arr.shape[1] == cols:
        return arr
    out = np.zeros((arr.shape[0], cols), dtype=arr.dtype)
    out[:, :arr.shape[1]] = arr
    return out
